# revision 6
# baseline (speedup 1.0000x reference)
"""GEMM + reduce-scatter (nn_GemmRSIntraNode) as a Bass/Tile kernel on 8 trn2 cores.

Full semantics: out = einsum('rmk,rnk->mn', input, weight).reshape(8, 1024, 4096)
with input [8, 8192, 1024] f32 and weight [8, 4096, 1024] f32.

Sharding choice: instead of mimicking the per-rank partial-GEMM +
reduce-scatter, each core c directly computes output rows
[c*1024:(c+1)*1024] of the reduced result:

    out_c = sum_{r,k} input[r, c*1024:(c+1)*1024, k] * weight[r, n, k]

i.e. a [1024, 8192] x [8192, 4096] GEMM per core where the contraction
axis is (r, k) flattened. The rank-sum IS the K-axis contraction, so no
cross-core communication is needed at all; the "reduce-scatter" is
absorbed into the GEMM. Inputs are pre-transposed host-side into
[K, M] / [K, N] layouts so the device kernel does only contiguous DMA
and matmuls.
"""

import os
from contextlib import ExitStack

import numpy as np

WS = 8
M = 8192
N = 4096
LK = 1024
K = WS * LK          # 8192 contraction (rank*local_k)
M_LOC = M // WS      # 1024 output rows per core
N_CORES = 8

# compute dtype: "float32r" (full-rate fp32 path), "float32" (4x slower,
# exact), or "bfloat16" (full rate, inputs rounded to bf16)
DTYPE = os.environ.get("BASS_KERNEL_DTYPE", "float32r")

_NC_CACHE = {}


def _build_nc(dt_name):
    import concourse.tile as tile
    from concourse import bacc, mybir

    f32 = mybir.dt.float32
    if dt_name == "bfloat16":
        io_dt = mybir.dt.bfloat16
        sb_dt = mybir.dt.bfloat16
    elif dt_name == "float32r":
        io_dt = f32
        sb_dt = mybir.dt.float32r
    elif dt_name == "float32":
        io_dt = f32
        sb_dt = f32
    else:
        raise ValueError(dt_name)
    # f32 -> f32r DMA is a "cast"; only gpsimd-initiated DMAs may cast
    load_engine = "gpsimd" if sb_dt != io_dt else "sync"

    # Blocking: keep an [K, M_RES] column block of A resident in SBUF and
    # stream B through it once per (m-block, n-block). PSUM macro tile is
    # MSUB x NSUB banks of [128, 512] fp32 accumulated over all 64 k-chunks.
    if dt_name == "bfloat16":
        M_RES = 1024   # whole per-core A resident (16MB bf16)
        NSUB = 1
    else:
        M_RES = 512    # half of A resident (16MB f32)
        NSUB = 2
    MSUB = M_RES // 128
    assert MSUB * NSUB == 8          # use all 8 PSUM banks
    NBW = NSUB * 512                 # n-block width
    NBLK = N // NBW
    MBLK = M_LOC // M_RES
    KC = K // 128                    # 64 k-chunks

    nc = bacc.Bacc("TRN2", target_bir_lowering=False, debug=False,
                   num_devices=N_CORES)
    a_d = nc.dram_tensor("a", [K, M_LOC], io_dt, kind="ExternalInput")
    b_d = nc.dram_tensor("b", [K, N], io_dt, kind="ExternalInput")
    o_d = nc.dram_tensor("o", [M_LOC, N], f32, kind="ExternalOutput")

    with tile.TileContext(nc) as tc, ExitStack() as ctx:
        apool = ctx.enter_context(tc.tile_pool(name="apool", bufs=KC + 4))
        bpool = ctx.enter_context(tc.tile_pool(name="bpool", bufs=4))
        cpool = ctx.enter_context(tc.tile_pool(name="cpool", bufs=8))
        pp = ctx.enter_context(tc.tile_pool(name="pp", bufs=8, space="PSUM"))

        for mb in range(MBLK):
            a_tiles = []
            for kc in range(KC):
                a_t = apool.tile([128, M_RES], sb_dt, name=f"a_{mb}_{kc}",
                                 tag="a")
                getattr(nc, load_engine).dma_start(
                    a_t[:],
                    a_d.ap()[kc * 128:(kc + 1) * 128,
                             mb * M_RES:(mb + 1) * M_RES])
                a_tiles.append(a_t)
            for nb in range(NBLK):
                psums = []
                for ms in range(MSUB):
                    row = []
                    for ns in range(NSUB):
                        p_t = pp.tile([128, 512], f32,
                                      name=f"p_{mb}_{nb}_{ms}_{ns}", tag="p")
                        row.append(p_t)
                    psums.append(row)
                for kc in range(KC):
                    b_t = bpool.tile([128, NBW], sb_dt,
                                     name=f"b_{mb}_{nb}_{kc}", tag="b")
                    getattr(nc, load_engine).dma_start(
                        b_t[:],
                        b_d.ap()[kc * 128:(kc + 1) * 128,
                                 nb * NBW:(nb + 1) * NBW])
                    for ms in range(MSUB):
                        lhsT = a_tiles[kc][:, ms * 128:(ms + 1) * 128]
                        for ns in range(NSUB):
                            rhs = b_t[:, ns * 512:(ns + 1) * 512]
                            nc.tensor.matmul(psums[ms][ns][:], lhsT, rhs,
                                             start=(kc == 0),
                                             stop=(kc == KC - 1))
                for ms in range(MSUB):
                    for ns in range(NSUB):
                        c_t = cpool.tile([128, 512], f32,
                                         name=f"c_{mb}_{nb}_{ms}_{ns}",
                                         tag="c")
                        nc.vector.tensor_copy(c_t[:], psums[ms][ns][:])
                        row0 = mb * M_RES + ms * 128
                        col0 = nb * NBW + ns * 512
                        nc.sync.dma_start(
                            o_d.ap()[row0:row0 + 128, col0:col0 + 512],
                            c_t[:])

    nc.compile()
    return nc


def get_nc(dt_name=None):
    dt_name = dt_name or DTYPE
    if dt_name not in _NC_CACHE:
        _NC_CACHE[dt_name] = _build_nc(dt_name)
    return _NC_CACHE[dt_name]


def make_in_maps(input, weight, dt_name=None):
    """Host-side shard + layout prep. Returns in_maps for cores 0..7."""
    dt_name = dt_name or DTYPE
    input = np.asarray(input, dtype=np.float32)
    weight = np.asarray(weight, dtype=np.float32)
    assert input.shape == (WS, M, LK), input.shape
    assert weight.shape == (WS, N, LK), weight.shape

    if dt_name == "bfloat16":
        import ml_dtypes
        np_dt = ml_dtypes.bfloat16
    else:
        np_dt = np.float32

    # B[r*LK + k, n] = weight[r, n, k]  -> [K, N]
    b_full = np.ascontiguousarray(
        weight.transpose(0, 2, 1).reshape(K, N).astype(np_dt))
    in_maps = []
    for c in range(N_CORES):
        # A_c[r*LK + k, m] = input[r, c*M_LOC + m, k]  -> [K, M_LOC]
        a_c = np.ascontiguousarray(
            input[:, c * M_LOC:(c + 1) * M_LOC, :]
            .transpose(0, 2, 1).reshape(K, M_LOC).astype(np_dt))
        in_maps.append({"a": a_c, "b": b_full})
    return in_maps


def kernel(input, weight):
    from concourse import bass_utils

    nc = get_nc()
    in_maps = make_in_maps(input, weight)
    res = bass_utils.run_bass_kernel_spmd(
        nc, in_maps, core_ids=list(range(N_CORES)))
    out = np.stack([res.results[c]["o"] for c in range(N_CORES)], axis=0)
    return out.astype(np.float32)
